# revision 18
# baseline (speedup 1.0000x reference)
"""Trainium2 Bass kernel for nn_Conv_39273180955616.

Computes, for X:(16,64,512,512) f32, K:(1,1,7,7), b:(1,1,1,1):
    out[n,c] = correlate2d(X[n,c], Keff, pad=3) + 49*b
where Keff = K.sum(axis=(0,1)).

Strategy: pure data parallel over the 1024 (n,c) planes -> 128 planes/core
on 8 cores.  Per plane, the 7x7 correlation runs on TensorE as
banded-Toeplitz matmuls in fp8(e4m3) DoubleRow mode: the h-dimension
contraction is a [128, 128] band matrix (7 diagonals of one kernel
column, dh = p - m) against an image block (rows on partitions).

v2 layout (column-parity banks, no duplicated shifted image):
Each image row is stored as [even-cols bank (259B) | odd-cols bank
(259B)] with the odd bank at +271 B from the even bank inside a
544 B block slot.  The 7 w-taps then pair up as DoubleRow slots with
pair stride +272 (legal: multiple of 16): for EVEN output columns the
dw-pairs are (-,0),(1,2),(3,4),(5,6); for ODD output columns
(0,1),(2,3),(4,5),(6,-).  Unused slots carry zero weights and read
in-bounds junk.  This halves input HBM traffic vs the shifted-copy
layout (2192 B/row-line vs 4160).

Each DR matmul merges TWO row-blocks via a 4-dim rhs AP
[[line,128],[272,2],[544,2],[1,256]] so N=512 and only 16 matmuls/plane
are needed (8 weight kinds: 4 dw-pairs x 2 column parities).  The
bottom 24 rows of 4 consecutive planes are packed block-diagonally
(27 rows x 4 on partitions) as in v1.

DMA: 4 planes batched per load (8768 B descriptors), per-plane stores
(4 KB descriptors), all on HWDGE: loads from SP (sync), stores from
ACT.  CRITICAL: every DMA's first AP dim is a multiple of 16 -- the
HWDGE sprays a transfer over (largest divisor <= 16 of the first dim)
SDMA engines, so 122 rows would collapse onto 2 engines.  PSUM is
evicted as fp16 with the bias added via 2-bank [128,1024] tiles
(1 ACT + 1 DVE instr per plane); the host upcasts to f32 and
unshuffles.  The first iteration's load is split per-plane so the
matmul stream starts after 1/4 of the load.

Measured: 494 us (vs 640 us baseline); TensorE-bound at 94% occupancy
with a gapless 216 ns/matmul stream (the warm 2.4 GHz hardware floor:
N=512 columns x 1 col/cycle + NX overhead; DoubleRow doubles the
contraction per cycle, not the streaming rate).
"""
import numpy as np
import ml_dtypes

import concourse.bass as bass
import concourse.tile as tile
from concourse import bacc, mybir
from concourse.bass_utils import run_bass_kernel_spmd

N_CORES = 8
H = 512
W = 512
LINE = 2192        # 16 front pad + 4 blocks x 544
BLK = 544          # per-block slot: [E 259 | gap | O 259 @ +271]
OBASE = 271        # odd bank offset inside a block slot
SSTRIDE = 272      # DoubleRow pair stride (multiple of 16)
GLINE = 560        # group line: 16 + 544
N_PLANES_TOTAL = 16 * 64
PLANES_PER_CORE = N_PLANES_TOTAL // N_CORES  # 128
GROUP = 4          # planes per batched load/store + bottom-tile merge
# input row start of blocks 1..3; block 0 is [3 zero rows, rows 0..124]
BSTARTS = (119, 241, 363)
KM = 122           # output rows per main tile
M_PAD = 128
# kinds 0-3: main EVEN-col dw pairs; 4-7: main ODD-col; 8-15: group same
EKINDS = [(None, 0), (1, 2), (3, 4), (5, 6)]
OKINDS = [(0, 1), (2, 3), (4, 5), (6, None)]
# slot-0 rhs byte offset within a block slot for each kind 0..7
KOFF = [15, 16, 17, 18, 16, 17, 18, 19]
N_KINDS = 16
WCOLS = N_KINDS * 2 * M_PAD

FP8 = ml_dtypes.float8_e4m3


def _band(Keff, Kk, M, dw):
    """[Kk, M_PAD] band matrix: mat[p, m] = Keff[p - m, dw]."""
    mat = np.zeros((Kk, M_PAD), np.float32)
    p = np.arange(Kk)[:, None]
    m = np.arange(M)[None, :]
    dh = p - m
    ok = (dh >= 0) & (dh < 7)
    sub = np.zeros((Kk, M), np.float32)
    sub[ok] = Keff[dh[ok], dw]
    mat[:, :M] = sub
    return mat


def _build_weight_pack(K8: np.ndarray) -> np.ndarray:
    """K8 (7,7) f32 (already e4m3-rounded) -> [128, WCOLS] fp8 lhsT pairs.

    Kind k at cols [k*256, +256): slot0 [0:128], slot1 [128:256].
    Kinds 0-7 = main (E pairs then O pairs); 8-15 = group block-diagonal.
    """
    wp = np.zeros((128, WCOLS), np.float32)
    kinds = EKINDS + OKINDS
    for k, (dw0, dw1) in enumerate(kinds):
        c0 = k * 2 * M_PAD
        if dw0 is not None:
            wp[:, c0:c0 + M_PAD] = _band(K8, 128, KM, dw0)
        if dw1 is not None:
            wp[:, c0 + M_PAD:c0 + 2 * M_PAD] = _band(K8, 128, KM, dw1)
    # group kinds: block-diagonal stack of GROUP (27 -> 24) bottom bands
    for k, (dw0, dw1) in enumerate(kinds):
        c0 = (8 + k) * 2 * M_PAD
        for s, dw in ((0, dw0), (1, dw1)):
            if dw is None:
                continue
            blk = _band(K8, 27, 24, dw)[:, :24]
            for g in range(GROUP):
                wp[27 * g:27 * g + 27,
                   c0 + s * M_PAD + 24 * g:c0 + s * M_PAD + 24 * g + 24] = blk
    return wp.astype(FP8)


_NC_CACHE = {}


def _get_module(n_planes: int):
    if n_planes in _NC_CACHE:
        return _NC_CACHE[n_planes]
    assert n_planes % GROUP == 0
    ng = n_planes // GROUP
    nc = bacc.Bacc("TRN2", target_bir_lowering=False, debug=False,
                   num_devices=N_CORES)
    xp = nc.dram_tensor("xp", [ng, 128, GROUP * LINE], mybir.dt.float8e4,
                        kind="ExternalInput")
    xg_d = nc.dram_tensor("xg", [ng, 112, GLINE], mybir.dt.float8e4,
                          kind="ExternalInput")
    wt = nc.dram_tensor("wt", [128, WCOLS], mybir.dt.float8e4,
                        kind="ExternalInput")
    bv = nc.dram_tensor("bv", [128, 1], mybir.dt.float32,
                        kind="ExternalInput")
    # partition-major store layouts (host unshuffles):
    # outm[p, r, P*1024 + par*512 + c*256 + q] holds plane p,
    # output row (2P+c)*122 + r, column 2q+par (fp16).
    outm = nc.dram_tensor("outm", [ng * GROUP, 128, 2048], mybir.dt.float16,
                          kind="ExternalOutput")
    # outb[i, 24g+m, par*256+q] holds plane 4i+g, row 488+m, col 2q+par.
    outb = nc.dram_tensor("outb", [ng, GROUP * 24, W], mybir.dt.float16,
                          kind="ExternalOutput")

    x_elems = 128 * GROUP * LINE
    g_elems = 112 * GLINE
    DR = mybir.MatmulPerfMode.DoubleRow

    with tile.TileContext(nc) as tc:
        with (
            tc.tile_pool(name="wp", bufs=1) as wpool,
            tc.tile_pool(name="xa", bufs=4) as xapool,
            tc.tile_pool(name="xg", bufs=3) as xgpool,
            tc.tile_pool(name="ps", bufs=4, space="PSUM") as pspool,
            tc.tile_pool(name="ob", bufs=8) as obpool,
            tc.tile_pool(name="og", bufs=3) as ogpool,
        ):
            wtile = wpool.tile([128, WCOLS], mybir.dt.float8e4)
            nc.sync.dma_start(wtile[:], wt.ap())
            btile = wpool.tile([128, 1], mybir.dt.float32)
            nc.sync.dma_start(btile[:], bv.ap())

            def lhsT(kind, Kk):
                c0 = kind * 2 * M_PAD
                return wtile[:Kk, c0:c0 + 2 * M_PAD].rearrange(
                    "k (two m) -> k two m", two=2)

            for i in range(ng):
                xa = xapool.tile([128, GROUP * LINE], mybir.dt.float8e4)
                if i == 0:
                    # first iteration: per-plane chunks so plane 0's
                    # matmuls start after 1/4 of the load (startup shave)
                    for g in range(GROUP):
                        nc.sync.dma_start(
                            xa[:, g * LINE:(g + 1) * LINE],
                            bass.AP(xp, g * LINE,
                                    [[GROUP * LINE, 128], [1, LINE]]))
                else:
                    nc.sync.dma_start(
                        xa[:], bass.AP(xp, i * x_elems,
                                       [[GROUP * LINE, 128],
                                        [1, GROUP * LINE]]))
                xgt = xgpool.tile([112, GLINE], mybir.dt.float8e4)
                nc.sync.dma_start(
                    xgt[:], bass.AP(xg_d, i * g_elems,
                                    [[GLINE, 112], [1, GLINE]]))
                xat = xa[:].tensor
                xgtt = xgt[:].tensor
                for g in range(GROUP):
                    ob = obpool.tile([128, 2048], mybir.dt.float16)
                    for P in range(2):
                        pt = pspool.tile([128, 1024], mybir.dt.float32,
                                         name="pt")
                        for par in range(2):
                            for j in range(4):
                                kind = par * 4 + j
                                rhs = bass.AP(
                                    xat,
                                    g * LINE + 2 * P * BLK + KOFF[kind],
                                    [[GROUP * LINE, 128], [SSTRIDE, 2],
                                     [BLK, 2], [1, 256]])
                                nc.tensor.matmul(
                                    pt[:, par * 512:par * 512 + 512],
                                    lhsT(kind, 128), rhs,
                                    start=(j == 0), stop=(j == 3),
                                    perf_mode=DR)
                        dst = ob[:, P * 1024:P * 1024 + 1024]
                        if P == 0:
                            nc.scalar.activation(
                                dst, pt[:, :],
                                mybir.ActivationFunctionType.Identity,
                                bias=btile[:, :], scale=1.0)
                        else:
                            nc.vector.tensor_scalar_add(
                                dst, pt[:, :], btile[:, :])

                    nc.scalar.dma_start(
                        bass.AP(outm, (i * GROUP + g) * 128 * 2048,
                                [[2048, 128], [1, 2048]]),
                        ob[:])

                    if g == GROUP - 2:
                        # merged bottom tiles of the 4 planes, scheduled
                        # before the last plane so eviction + store overlap
                        ptg = pspool.tile([128, 1024], mybir.dt.float32,
                                          name="pt")
                        for par in range(2):
                            for j in range(4):
                                kind = 8 + par * 4 + j
                                rhs = bass.AP(
                                    xgtt, KOFF[kind - 8],
                                    [[GLINE, GROUP * 27], [SSTRIDE, 2],
                                     [1, 256]])
                                nc.tensor.matmul(
                                    ptg[:, par * 256:par * 256 + 256],
                                    lhsT(kind, GROUP * 27), rhs,
                                    start=(j == 0), stop=(j == 3),
                                    perf_mode=DR)
                        og = ogpool.tile([GROUP * 24, W], mybir.dt.float16)
                        nc.scalar.activation(
                            og[:], ptg[:GROUP * 24, :512],
                            mybir.ActivationFunctionType.Identity,
                            bias=btile[:GROUP * 24, :], scale=1.0)
                        nc.scalar.dma_start(
                            bass.AP(outb, i * GROUP * 24 * W,
                                    [[W, GROUP * 24], [1, W]]),
                            og[:])

    nc.compile()
    _NC_CACHE[n_planes] = nc
    return nc


def _prep_inputs(X, K, b, n_cores=N_CORES):
    Keff = np.asarray(K, np.float32).sum(axis=(0, 1))
    K8 = Keff.astype(FP8).astype(np.float32)
    wt = _build_weight_pack(K8)
    bias = np.float32(np.asarray(b).reshape(-1)[0]) * np.float32(K.size)
    bv = np.full((128, 1), bias, np.float32)

    Xr = np.asarray(X, np.float32).reshape(-1, H, W)
    n_total = Xr.shape[0]
    per = n_total // n_cores
    ng_total = n_total // GROUP
    # zero-padded fp8 planes, column-parity split
    P8 = np.zeros((n_total, H + 6, W + 6), FP8)
    P8[:, 3:3 + H, 3:3 + W] = Xr.astype(FP8)
    E = P8[:, :, 1::2]   # even image cols (259)
    O = P8[:, :, 0::2]   # odd image cols (259)
    Xline = np.zeros((n_total, 128, LINE), FP8)
    for t in range(4):
        if t == 0:
            re, ro = E[:, 0:128], O[:, 0:128]
        else:
            s = BSTARTS[t - 1] + 3
            re, ro = E[:, s:s + 128], O[:, s:s + 128]
        base = 16 + BLK * t
        Xline[:, :, base:base + 259] = re
        Xline[:, :, base + OBASE:base + OBASE + 259] = ro
    Xp = (Xline.reshape(ng_total, GROUP, 128, LINE)
          .transpose(0, 2, 1, 3).reshape(ng_total, 128, GROUP * LINE))
    # group bottom blocks: P-rows 488..514 (image rows 485..511)
    Xg = np.zeros((ng_total, 112, GLINE), FP8)
    Xg[:, :108, 16:16 + 259] = (
        E[:, 488:515].reshape(ng_total, GROUP * 27, 259))
    Xg[:, :108, 287:287 + 259] = (
        O[:, 488:515].reshape(ng_total, GROUP * 27, 259))
    ng = per // GROUP
    in_maps = [
        {"xp": Xp[i * ng:(i + 1) * ng],
         "xg": Xg[i * ng:(i + 1) * ng],
         "wt": wt, "bv": bv}
        for i in range(n_cores)
    ]
    return in_maps, per


def kernel(X, K, b):
    in_maps, per = _prep_inputs(X, K, b)
    nc = _get_module(per)
    res = run_bass_kernel_spmd(nc, in_maps, list(range(N_CORES)))
    outm = np.concatenate([res.results[i]["outm"] for i in range(N_CORES)],
                          axis=0)  # [n, 128, 2048] fp16
    outb = np.concatenate([res.results[i]["outb"] for i in range(N_CORES)],
                          axis=0)  # [ng, 96, 512] fp16
    n_total = outm.shape[0]
    ng = n_total // GROUP
    full = np.empty((n_total, H, W), np.float32)
    # outm[p, m, P, par, c, q] -> plane p, row 122*(2P+c)+m, col 2q+par
    o = outm[:, :KM].reshape(n_total, KM, 2, 2, 2, 256)
    full[:, :4 * KM] = (o.transpose(0, 2, 4, 1, 5, 3)
                        .reshape(n_total, 4 * KM, W).astype(np.float32))
    # outb[i, g, m, par, q] -> plane 4i+g, row 488+m, col 2q+par
    ob = outb.reshape(ng, GROUP, 24, 2, 256)
    full[:, 4 * KM:] = (ob.transpose(0, 1, 2, 4, 3)
                        .reshape(n_total, 24, W).astype(np.float32))
    return full.reshape(np.asarray(X).shape)


# revision 20
# speedup vs baseline: 1.0087x; 1.0087x over previous
"""Trainium2 Bass kernel for nn_Conv_39273180955616.

Computes, for X:(16,64,512,512) f32, K:(1,1,7,7), b:(1,1,1,1):
    out[n,c] = correlate2d(X[n,c], Keff, pad=3) + 49*b
where Keff = K.sum(axis=(0,1)).

Strategy: pure data parallel over the 1024 (n,c) planes -> 128 planes/core
on 8 cores.  Per plane, the 7x7 correlation runs on TensorE as
banded-Toeplitz matmuls in fp8(e4m3) DoubleRow mode: the h-dimension
contraction is a [128, 128] band matrix (7 diagonals of one kernel
column, dh = p - m) against an image block (rows on partitions).

v2 layout (column-parity banks, no duplicated shifted image):
Each image row is stored as [even-cols bank (259B) | odd-cols bank
(259B)] with the odd bank at +271 B from the even bank inside a
544 B block slot.  The 7 w-taps then pair up as DoubleRow slots with
pair stride +272 (legal: multiple of 16): for EVEN output columns the
dw-pairs are (-,0),(1,2),(3,4),(5,6); for ODD output columns
(0,1),(2,3),(4,5),(6,-).  Unused slots carry zero weights and read
in-bounds junk.  This halves input HBM traffic vs the shifted-copy
layout (2192 B/row-line vs 4160).

Each DR matmul merges TWO row-blocks via a 4-dim rhs AP
[[line,128],[272,2],[544,2],[1,256]] so N=512 and only 16 matmuls/plane
are needed (8 weight kinds: 4 dw-pairs x 2 column parities).  The
bottom 24 rows of 4 consecutive planes are packed block-diagonally
(27 rows x 4 on partitions) as in v1.

DMA: 4 planes batched per load (8768 B descriptors), per-plane stores
(4 KB descriptors), all on HWDGE: loads from SP (sync), stores from
ACT.  CRITICAL: every DMA's first AP dim is a multiple of 16 -- the
HWDGE sprays a transfer over (largest divisor <= 16 of the first dim)
SDMA engines, so 122 rows would collapse onto 2 engines.  PSUM is
evicted as fp16 with the bias added via 2-bank [128,1024] tiles
(1 ACT + 1 DVE instr per plane); the host upcasts to f32 and
unshuffles.  The first iteration's load is split per-plane so the
matmul stream starts after 1/4 of the load.

Measured: 494 us (vs 640 us baseline); TensorE-bound at 94% occupancy
with a gapless 216 ns/matmul stream (the warm 2.4 GHz hardware floor:
N=512 columns x 1 col/cycle + NX overhead; DoubleRow doubles the
contraction per cycle, not the streaming rate).
"""
import numpy as np
import ml_dtypes

import concourse.bass as bass
import concourse.tile as tile
from concourse import bacc, mybir
from concourse.bass_utils import run_bass_kernel_spmd

N_CORES = 8
H = 512
W = 512
LINE = 2192        # 16 front pad + 4 blocks x 544
BLK = 544          # per-block slot: [E 259 | gap | O 259 @ +271]
OBASE = 271        # odd bank offset inside a block slot
SSTRIDE = 272      # DoubleRow pair stride (multiple of 16)
GLINE = 560        # group line: 16 + 544
N_PLANES_TOTAL = 16 * 64
PLANES_PER_CORE = N_PLANES_TOTAL // N_CORES  # 128
GROUP = 4          # planes per batched load/store + bottom-tile merge
# input row start of blocks 1..3; block 0 is [3 zero rows, rows 0..124]
BSTARTS = (119, 241, 363)
KM = 122           # output rows per main tile
M_PAD = 128
# kinds 0-3: main EVEN-col dw pairs; 4-7: main ODD-col; 8-15: group same
EKINDS = [(None, 0), (1, 2), (3, 4), (5, 6)]
OKINDS = [(0, 1), (2, 3), (4, 5), (6, None)]
# slot-0 rhs byte offset within a block slot for each kind 0..7
KOFF = [15, 16, 17, 18, 16, 17, 18, 19]
N_KINDS = 16
WCOLS = N_KINDS * 2 * M_PAD

FP8 = ml_dtypes.float8_e4m3


def _band(Keff, Kk, M, dw):
    """[Kk, M_PAD] band matrix: mat[p, m] = Keff[p - m, dw]."""
    mat = np.zeros((Kk, M_PAD), np.float32)
    p = np.arange(Kk)[:, None]
    m = np.arange(M)[None, :]
    dh = p - m
    ok = (dh >= 0) & (dh < 7)
    sub = np.zeros((Kk, M), np.float32)
    sub[ok] = Keff[dh[ok], dw]
    mat[:, :M] = sub
    return mat


def _build_weight_pack(K8: np.ndarray) -> np.ndarray:
    """K8 (7,7) f32 (already e4m3-rounded) -> [128, WCOLS] fp8 lhsT pairs.

    Kind k at cols [k*256, +256): slot0 [0:128], slot1 [128:256].
    Kinds 0-7 = main (E pairs then O pairs); 8-15 = group block-diagonal.
    """
    wp = np.zeros((128, WCOLS), np.float32)
    kinds = EKINDS + OKINDS
    for k, (dw0, dw1) in enumerate(kinds):
        c0 = k * 2 * M_PAD
        if dw0 is not None:
            wp[:, c0:c0 + M_PAD] = _band(K8, 128, KM, dw0)
        if dw1 is not None:
            wp[:, c0 + M_PAD:c0 + 2 * M_PAD] = _band(K8, 128, KM, dw1)
    # group kinds: block-diagonal stack of GROUP (27 -> 24) bottom bands
    for k, (dw0, dw1) in enumerate(kinds):
        c0 = (8 + k) * 2 * M_PAD
        for s, dw in ((0, dw0), (1, dw1)):
            if dw is None:
                continue
            blk = _band(K8, 27, 24, dw)[:, :24]
            for g in range(GROUP):
                wp[27 * g:27 * g + 27,
                   c0 + s * M_PAD + 24 * g:c0 + s * M_PAD + 24 * g + 24] = blk
    return wp.astype(FP8)


_NC_CACHE = {}


def _get_module(n_planes: int):
    if n_planes in _NC_CACHE:
        return _NC_CACHE[n_planes]
    assert n_planes % GROUP == 0
    ng = n_planes // GROUP
    nc = bacc.Bacc("TRN2", target_bir_lowering=False, debug=False,
                   num_devices=N_CORES)
    xp = nc.dram_tensor("xp", [ng, 128, GROUP * LINE], mybir.dt.float8e4,
                        kind="ExternalInput")
    xg_d = nc.dram_tensor("xg", [ng, 112, GLINE], mybir.dt.float8e4,
                          kind="ExternalInput")
    wt = nc.dram_tensor("wt", [128, WCOLS], mybir.dt.float8e4,
                        kind="ExternalInput")
    bv = nc.dram_tensor("bv", [128, 1], mybir.dt.float32,
                        kind="ExternalInput")
    # partition-major store layouts (host unshuffles):
    # outm[p, r, P*1024 + par*512 + c*256 + q] holds plane p,
    # output row (2P+c)*122 + r, column 2q+par (fp16).
    outm = nc.dram_tensor("outm", [ng * GROUP, 128, 2048], mybir.dt.float16,
                          kind="ExternalOutput")
    # outb[i, 24g+m, par*256+q] holds plane 4i+g, row 488+m, col 2q+par.
    outb = nc.dram_tensor("outb", [ng, GROUP * 24, W], mybir.dt.float16,
                          kind="ExternalOutput")

    x_elems = 128 * GROUP * LINE
    g_elems = 112 * GLINE
    DR = mybir.MatmulPerfMode.DoubleRow

    with tile.TileContext(nc) as tc:
        with (
            tc.tile_pool(name="wp", bufs=1) as wpool,
            tc.tile_pool(name="xa", bufs=5) as xapool,
            tc.tile_pool(name="xg", bufs=3) as xgpool,
            tc.tile_pool(name="ps", bufs=4, space="PSUM") as pspool,
            tc.tile_pool(name="ob", bufs=8) as obpool,
            tc.tile_pool(name="og", bufs=3) as ogpool,
        ):
            # weights + bias load on the ACT ring so they run in parallel
            # with the first xa chunks on the SP ring (both gate the first
            # matmul); main kinds (cols 0:2048) first, group kinds second
            wtile = wpool.tile([128, WCOLS], mybir.dt.float8e4)
            nc.scalar.dma_start(
                wtile[:, 0:WCOLS // 2],
                bass.AP(wt, 0, [[WCOLS, 128], [1, WCOLS // 2]]))
            nc.scalar.dma_start(
                wtile[:, WCOLS // 2:WCOLS],
                bass.AP(wt, WCOLS // 2, [[WCOLS, 128], [1, WCOLS // 2]]))
            btile = wpool.tile([128, 1], mybir.dt.float32)
            nc.scalar.dma_start(btile[:], bv.ap())

            def lhsT(kind, Kk):
                c0 = kind * 2 * M_PAD
                return wtile[:Kk, c0:c0 + 2 * M_PAD].rearrange(
                    "k (two m) -> k two m", two=2)

            for i in range(ng):
                xa = xapool.tile([128, GROUP * LINE], mybir.dt.float8e4)
                if i == 0:
                    # first iteration: per-plane chunks so plane 0's
                    # matmuls start after 1/4 of the load (startup shave)
                    for g in range(GROUP):
                        nc.sync.dma_start(
                            xa[:, g * LINE:(g + 1) * LINE],
                            bass.AP(xp, g * LINE,
                                    [[GROUP * LINE, 128], [1, LINE]]))
                else:
                    nc.sync.dma_start(
                        xa[:], bass.AP(xp, i * x_elems,
                                       [[GROUP * LINE, 128],
                                        [1, GROUP * LINE]]))
                xgt = xgpool.tile([112, GLINE], mybir.dt.float8e4)
                nc.sync.dma_start(
                    xgt[:], bass.AP(xg_d, i * g_elems,
                                    [[GLINE, 112], [1, GLINE]]))
                xat = xa[:].tensor
                xgtt = xgt[:].tensor
                for g in range(GROUP):
                    ob = obpool.tile([128, 2048], mybir.dt.float16)
                    for P in range(2):
                        pt = pspool.tile([128, 1024], mybir.dt.float32,
                                         name="pt")
                        for par in range(2):
                            for j in range(4):
                                kind = par * 4 + j
                                rhs = bass.AP(
                                    xat,
                                    g * LINE + 2 * P * BLK + KOFF[kind],
                                    [[GROUP * LINE, 128], [SSTRIDE, 2],
                                     [BLK, 2], [1, 256]])
                                nc.tensor.matmul(
                                    pt[:, par * 512:par * 512 + 512],
                                    lhsT(kind, 128), rhs,
                                    start=(j == 0), stop=(j == 3),
                                    perf_mode=DR)
                        dst = ob[:, P * 1024:P * 1024 + 1024]
                        if P == 0:
                            nc.scalar.activation(
                                dst, pt[:, :],
                                mybir.ActivationFunctionType.Identity,
                                bias=btile[:, :], scale=1.0)
                        else:
                            nc.vector.tensor_scalar_add(
                                dst, pt[:, :], btile[:, :])

                    nc.scalar.dma_start(
                        bass.AP(outm, (i * GROUP + g) * 128 * 2048,
                                [[2048, 128], [1, 2048]]),
                        ob[:])

                    if g == GROUP - 2:
                        # merged bottom tiles of the 4 planes, scheduled
                        # before the last plane so eviction + store overlap
                        ptg = pspool.tile([128, 1024], mybir.dt.float32,
                                          name="pt")
                        for par in range(2):
                            for j in range(4):
                                kind = 8 + par * 4 + j
                                rhs = bass.AP(
                                    xgtt, KOFF[kind - 8],
                                    [[GLINE, GROUP * 27], [SSTRIDE, 2],
                                     [1, 256]])
                                nc.tensor.matmul(
                                    ptg[:, par * 256:par * 256 + 256],
                                    lhsT(kind, GROUP * 27), rhs,
                                    start=(j == 0), stop=(j == 3),
                                    perf_mode=DR)
                        og = ogpool.tile([GROUP * 24, W], mybir.dt.float16)
                        nc.scalar.activation(
                            og[:], ptg[:GROUP * 24, :512],
                            mybir.ActivationFunctionType.Identity,
                            bias=btile[:GROUP * 24, :], scale=1.0)
                        nc.scalar.dma_start(
                            bass.AP(outb, i * GROUP * 24 * W,
                                    [[W, GROUP * 24], [1, W]]),
                            og[:])

    nc.compile()
    _NC_CACHE[n_planes] = nc
    return nc


def _prep_inputs(X, K, b, n_cores=N_CORES):
    Keff = np.asarray(K, np.float32).sum(axis=(0, 1))
    K8 = Keff.astype(FP8).astype(np.float32)
    wt = _build_weight_pack(K8)
    bias = np.float32(np.asarray(b).reshape(-1)[0]) * np.float32(K.size)
    bv = np.full((128, 1), bias, np.float32)

    Xr = np.asarray(X, np.float32).reshape(-1, H, W)
    n_total = Xr.shape[0]
    per = n_total // n_cores
    ng_total = n_total // GROUP
    # zero-padded fp8 planes, column-parity split
    P8 = np.zeros((n_total, H + 6, W + 6), FP8)
    P8[:, 3:3 + H, 3:3 + W] = Xr.astype(FP8)
    E = P8[:, :, 1::2]   # even image cols (259)
    O = P8[:, :, 0::2]   # odd image cols (259)
    Xline = np.zeros((n_total, 128, LINE), FP8)
    for t in range(4):
        if t == 0:
            re, ro = E[:, 0:128], O[:, 0:128]
        else:
            s = BSTARTS[t - 1] + 3
            re, ro = E[:, s:s + 128], O[:, s:s + 128]
        base = 16 + BLK * t
        Xline[:, :, base:base + 259] = re
        Xline[:, :, base + OBASE:base + OBASE + 259] = ro
    Xp = (Xline.reshape(ng_total, GROUP, 128, LINE)
          .transpose(0, 2, 1, 3).reshape(ng_total, 128, GROUP * LINE))
    # group bottom blocks: P-rows 488..514 (image rows 485..511)
    Xg = np.zeros((ng_total, 112, GLINE), FP8)
    Xg[:, :108, 16:16 + 259] = (
        E[:, 488:515].reshape(ng_total, GROUP * 27, 259))
    Xg[:, :108, 287:287 + 259] = (
        O[:, 488:515].reshape(ng_total, GROUP * 27, 259))
    ng = per // GROUP
    in_maps = [
        {"xp": Xp[i * ng:(i + 1) * ng],
         "xg": Xg[i * ng:(i + 1) * ng],
         "wt": wt, "bv": bv}
        for i in range(n_cores)
    ]
    return in_maps, per


def kernel(X, K, b):
    in_maps, per = _prep_inputs(X, K, b)
    nc = _get_module(per)
    res = run_bass_kernel_spmd(nc, in_maps, list(range(N_CORES)))
    outm = np.concatenate([res.results[i]["outm"] for i in range(N_CORES)],
                          axis=0)  # [n, 128, 2048] fp16
    outb = np.concatenate([res.results[i]["outb"] for i in range(N_CORES)],
                          axis=0)  # [ng, 96, 512] fp16
    n_total = outm.shape[0]
    ng = n_total // GROUP
    full = np.empty((n_total, H, W), np.float32)
    # outm[p, m, P, par, c, q] -> plane p, row 122*(2P+c)+m, col 2q+par
    o = outm[:, :KM].reshape(n_total, KM, 2, 2, 2, 256)
    full[:, :4 * KM] = (o.transpose(0, 2, 4, 1, 5, 3)
                        .reshape(n_total, 4 * KM, W).astype(np.float32))
    # outb[i, g, m, par, q] -> plane 4i+g, row 488+m, col 2q+par
    ob = outb.reshape(ng, GROUP, 24, 2, 256)
    full[:, 4 * KM:] = (ob.transpose(0, 1, 2, 4, 3)
                        .reshape(n_total, 24, W).astype(np.float32))
    return full.reshape(np.asarray(X).shape)


# revision 21
# speedup vs baseline: 1.0132x; 1.0044x over previous
"""Trainium2 Bass kernel for nn_Conv_39273180955616.

Computes, for X:(16,64,512,512) f32, K:(1,1,7,7), b:(1,1,1,1):
    out[n,c] = correlate2d(X[n,c], Keff, pad=3) + 49*b
where Keff = K.sum(axis=(0,1)).

Strategy: pure data parallel over the 1024 (n,c) planes -> 128 planes/core
on 8 cores.  Per plane, the 7x7 correlation runs on TensorE as
banded-Toeplitz matmuls in fp8(e4m3) DoubleRow mode: the h-dimension
contraction is a [128, 128] band matrix (7 diagonals of one kernel
column, dh = p - m) against an image block (rows on partitions).

v2 layout (column-parity banks, no duplicated shifted image):
Each image row is stored as [even-cols bank (259B) | odd-cols bank
(259B)] with the odd bank at +271 B from the even bank inside a
544 B block slot.  The 7 w-taps then pair up as DoubleRow slots with
pair stride +272 (legal: multiple of 16): for EVEN output columns the
dw-pairs are (-,0),(1,2),(3,4),(5,6); for ODD output columns
(0,1),(2,3),(4,5),(6,-).  Unused slots carry zero weights and read
in-bounds junk.  This halves input HBM traffic vs the shifted-copy
layout (2192 B/row-line vs 4160).

Each DR matmul merges TWO row-blocks via a 4-dim rhs AP
[[line,128],[272,2],[544,2],[1,256]] so N=512 and only 16 matmuls/plane
are needed (8 weight kinds: 4 dw-pairs x 2 column parities).  The
bottom 24 rows of 4 consecutive planes are packed block-diagonally
(27 rows x 4 on partitions) as in v1.

DMA: 4 planes batched per load (8768 B descriptors), per-plane stores
(4 KB descriptors), all on HWDGE: loads from SP (sync), stores from
ACT.  CRITICAL: every DMA's first AP dim is a multiple of 16 -- the
HWDGE sprays a transfer over (largest divisor <= 16 of the first dim)
SDMA engines, so 122 rows would collapse onto 2 engines.  PSUM is
evicted as fp16 with the bias added via 2-bank [128,1024] tiles
(1 ACT + 1 DVE instr per plane); the host upcasts to f32 and
unshuffles.  The first iteration's load is split per-plane so the
matmul stream starts after 1/4 of the load.

Measured: 494 us (vs 640 us baseline); TensorE-bound at 94% occupancy
with a gapless 216 ns/matmul stream (the warm 2.4 GHz hardware floor:
N=512 columns x 1 col/cycle + NX overhead; DoubleRow doubles the
contraction per cycle, not the streaming rate).
"""
import numpy as np
import ml_dtypes

import concourse.bass as bass
import concourse.tile as tile
from concourse import bacc, mybir
from concourse.bass_utils import run_bass_kernel_spmd

N_CORES = 8
H = 512
W = 512
LINE = 2192        # 16 front pad + 4 blocks x 544
BLK = 544          # per-block slot: [E 259 | gap | O 259 @ +271]
OBASE = 271        # odd bank offset inside a block slot
SSTRIDE = 272      # DoubleRow pair stride (multiple of 16)
GLINE = 560        # group line: 16 + 544
N_PLANES_TOTAL = 16 * 64
PLANES_PER_CORE = N_PLANES_TOTAL // N_CORES  # 128
GROUP = 4          # planes per batched load/store + bottom-tile merge
# input row start of blocks 1..3; block 0 is [3 zero rows, rows 0..124]
BSTARTS = (119, 241, 363)
KM = 122           # output rows per main tile
M_PAD = 128
# kinds 0-3: main EVEN-col dw pairs; 4-7: main ODD-col; 8-15: group same
EKINDS = [(None, 0), (1, 2), (3, 4), (5, 6)]
OKINDS = [(0, 1), (2, 3), (4, 5), (6, None)]
# slot-0 rhs byte offset within a block slot for each kind 0..7
KOFF = [15, 16, 17, 18, 16, 17, 18, 19]
N_KINDS = 16
WCOLS = N_KINDS * 2 * M_PAD

FP8 = ml_dtypes.float8_e4m3


def _band(Keff, Kk, M, dw):
    """[Kk, M_PAD] band matrix: mat[p, m] = Keff[p - m, dw]."""
    mat = np.zeros((Kk, M_PAD), np.float32)
    p = np.arange(Kk)[:, None]
    m = np.arange(M)[None, :]
    dh = p - m
    ok = (dh >= 0) & (dh < 7)
    sub = np.zeros((Kk, M), np.float32)
    sub[ok] = Keff[dh[ok], dw]
    mat[:, :M] = sub
    return mat


def _build_weight_pack(K8: np.ndarray) -> np.ndarray:
    """K8 (7,7) f32 (already e4m3-rounded) -> [128, WCOLS] fp8 lhsT pairs.

    Kind k at cols [k*256, +256): slot0 [0:128], slot1 [128:256].
    Kinds 0-7 = main (E pairs then O pairs); 8-15 = group block-diagonal.
    """
    wp = np.zeros((128, WCOLS), np.float32)
    kinds = EKINDS + OKINDS
    for k, (dw0, dw1) in enumerate(kinds):
        c0 = k * 2 * M_PAD
        if dw0 is not None:
            wp[:, c0:c0 + M_PAD] = _band(K8, 128, KM, dw0)
        if dw1 is not None:
            wp[:, c0 + M_PAD:c0 + 2 * M_PAD] = _band(K8, 128, KM, dw1)
    # group kinds: block-diagonal stack of GROUP (27 -> 24) bottom bands
    for k, (dw0, dw1) in enumerate(kinds):
        c0 = (8 + k) * 2 * M_PAD
        for s, dw in ((0, dw0), (1, dw1)):
            if dw is None:
                continue
            blk = _band(K8, 27, 24, dw)[:, :24]
            for g in range(GROUP):
                wp[27 * g:27 * g + 27,
                   c0 + s * M_PAD + 24 * g:c0 + s * M_PAD + 24 * g + 24] = blk
    return wp.astype(FP8)


_NC_CACHE = {}


def _get_module(n_planes: int):
    if n_planes in _NC_CACHE:
        return _NC_CACHE[n_planes]
    assert n_planes % GROUP == 0
    ng = n_planes // GROUP
    nc = bacc.Bacc("TRN2", target_bir_lowering=False, debug=False,
                   num_devices=N_CORES)
    xp = nc.dram_tensor("xp", [ng, 128, GROUP * LINE], mybir.dt.float8e4,
                        kind="ExternalInput")
    xg_d = nc.dram_tensor("xg", [ng, 112, GLINE], mybir.dt.float8e4,
                          kind="ExternalInput")
    wt = nc.dram_tensor("wt", [128, WCOLS], mybir.dt.float8e4,
                        kind="ExternalInput")
    bv = nc.dram_tensor("bv", [128, 1], mybir.dt.float32,
                        kind="ExternalInput")
    # partition-major store layouts (host unshuffles):
    # outm[p, r, P*1024 + par*512 + c*256 + q] holds plane p,
    # output row (2P+c)*122 + r, column 2q+par (fp16).
    outm = nc.dram_tensor("outm", [ng * GROUP, 128, 2048], mybir.dt.float16,
                          kind="ExternalOutput")
    # outb[i, 24g+m, par*256+q] holds plane 4i+g, row 488+m, col 2q+par.
    outb = nc.dram_tensor("outb", [ng, GROUP * 24, W], mybir.dt.float16,
                          kind="ExternalOutput")

    x_elems = 128 * GROUP * LINE
    g_elems = 112 * GLINE
    DR = mybir.MatmulPerfMode.DoubleRow

    with tile.TileContext(nc) as tc:
        with (
            tc.tile_pool(name="wp", bufs=1) as wpool,
            tc.tile_pool(name="xa", bufs=5) as xapool,
            tc.tile_pool(name="xg", bufs=3) as xgpool,
            tc.tile_pool(name="ps", bufs=4, space="PSUM") as pspool,
            tc.tile_pool(name="ob", bufs=8) as obpool,
            tc.tile_pool(name="og", bufs=3) as ogpool,
        ):
            # weights + bias load on the ACT ring so they run in parallel
            # with the first xa chunks on the SP ring (both gate the first
            # matmul); main kinds (cols 0:2048) first, group kinds second
            wtile = wpool.tile([128, WCOLS], mybir.dt.float8e4)
            nc.scalar.dma_start(
                wtile[:, 0:WCOLS // 2],
                bass.AP(wt, 0, [[WCOLS, 128], [1, WCOLS // 2]]))
            nc.scalar.dma_start(
                wtile[:, WCOLS // 2:WCOLS],
                bass.AP(wt, WCOLS // 2, [[WCOLS, 128], [1, WCOLS // 2]]))
            btile = wpool.tile([128, 1], mybir.dt.float32)
            nc.scalar.dma_start(btile[:], bv.ap())

            def lhsT(kind, Kk):
                c0 = kind * 2 * M_PAD
                return wtile[:Kk, c0:c0 + 2 * M_PAD].rearrange(
                    "k (two m) -> k two m", two=2)

            # HAM pre-warm: the PE clock sits at 1.2 GHz until ~3.4us of
            # sustained activity.  Issue dummy matmuls on a zeroed scratch
            # tile during the initial load latency so the real stream
            # starts at the warm 2.4 GHz rate (saves ~2.8us of cold MMs).
            warm = wpool.tile([128, 544], mybir.dt.float8e4)
            nc.gpsimd.memset(warm[:], 0)
            wpt = pspool.tile([128, 1024], mybir.dt.float32, name="pt")
            wlhsT = warm[:, 0:256].rearrange("k (two m) -> k two m", two=2)
            wrhs = bass.AP(warm[:].tensor, 0,
                           [[544, 128], [16, 2], [1, 256]])
            for _ in range(16):
                nc.tensor.matmul(wpt[:, 0:256], wlhsT, wrhs,
                                 start=True, stop=True, perf_mode=DR)

            for i in range(ng):
                xa = xapool.tile([128, GROUP * LINE], mybir.dt.float8e4)
                if i == 0:
                    # first iteration: per-plane chunks so plane 0's
                    # matmuls start after 1/4 of the load (startup shave)
                    for g in range(GROUP):
                        nc.sync.dma_start(
                            xa[:, g * LINE:(g + 1) * LINE],
                            bass.AP(xp, g * LINE,
                                    [[GROUP * LINE, 128], [1, LINE]]))
                else:
                    nc.sync.dma_start(
                        xa[:], bass.AP(xp, i * x_elems,
                                       [[GROUP * LINE, 128],
                                        [1, GROUP * LINE]]))
                xgt = xgpool.tile([112, GLINE], mybir.dt.float8e4)
                nc.sync.dma_start(
                    xgt[:], bass.AP(xg_d, i * g_elems,
                                    [[GLINE, 112], [1, GLINE]]))
                xat = xa[:].tensor
                xgtt = xgt[:].tensor
                for g in range(GROUP):
                    ob = obpool.tile([128, 2048], mybir.dt.float16)
                    for P in range(2):
                        pt = pspool.tile([128, 1024], mybir.dt.float32,
                                         name="pt")
                        for par in range(2):
                            for j in range(4):
                                kind = par * 4 + j
                                rhs = bass.AP(
                                    xat,
                                    g * LINE + 2 * P * BLK + KOFF[kind],
                                    [[GROUP * LINE, 128], [SSTRIDE, 2],
                                     [BLK, 2], [1, 256]])
                                nc.tensor.matmul(
                                    pt[:, par * 512:par * 512 + 512],
                                    lhsT(kind, 128), rhs,
                                    start=(j == 0), stop=(j == 3),
                                    perf_mode=DR)
                        dst = ob[:, P * 1024:P * 1024 + 1024]
                        if P == 0:
                            nc.scalar.activation(
                                dst, pt[:, :],
                                mybir.ActivationFunctionType.Identity,
                                bias=btile[:, :], scale=1.0)
                        else:
                            nc.vector.tensor_scalar_add(
                                dst, pt[:, :], btile[:, :])

                    nc.scalar.dma_start(
                        bass.AP(outm, (i * GROUP + g) * 128 * 2048,
                                [[2048, 128], [1, 2048]]),
                        ob[:])

                    if g == GROUP - 2:
                        # merged bottom tiles of the 4 planes, scheduled
                        # before the last plane so eviction + store overlap
                        ptg = pspool.tile([128, 1024], mybir.dt.float32,
                                          name="pt")
                        for par in range(2):
                            for j in range(4):
                                kind = 8 + par * 4 + j
                                rhs = bass.AP(
                                    xgtt, KOFF[kind - 8],
                                    [[GLINE, GROUP * 27], [SSTRIDE, 2],
                                     [1, 256]])
                                nc.tensor.matmul(
                                    ptg[:, par * 256:par * 256 + 256],
                                    lhsT(kind, GROUP * 27), rhs,
                                    start=(j == 0), stop=(j == 3),
                                    perf_mode=DR)
                        og = ogpool.tile([GROUP * 24, W], mybir.dt.float16)
                        nc.scalar.activation(
                            og[:], ptg[:GROUP * 24, :512],
                            mybir.ActivationFunctionType.Identity,
                            bias=btile[:GROUP * 24, :], scale=1.0)
                        nc.scalar.dma_start(
                            bass.AP(outb, i * GROUP * 24 * W,
                                    [[W, GROUP * 24], [1, W]]),
                            og[:])

    nc.compile()
    _NC_CACHE[n_planes] = nc
    return nc


def _prep_inputs(X, K, b, n_cores=N_CORES):
    Keff = np.asarray(K, np.float32).sum(axis=(0, 1))
    K8 = Keff.astype(FP8).astype(np.float32)
    wt = _build_weight_pack(K8)
    bias = np.float32(np.asarray(b).reshape(-1)[0]) * np.float32(K.size)
    bv = np.full((128, 1), bias, np.float32)

    Xr = np.asarray(X, np.float32).reshape(-1, H, W)
    n_total = Xr.shape[0]
    per = n_total // n_cores
    ng_total = n_total // GROUP
    # zero-padded fp8 planes, column-parity split
    P8 = np.zeros((n_total, H + 6, W + 6), FP8)
    P8[:, 3:3 + H, 3:3 + W] = Xr.astype(FP8)
    E = P8[:, :, 1::2]   # even image cols (259)
    O = P8[:, :, 0::2]   # odd image cols (259)
    Xline = np.zeros((n_total, 128, LINE), FP8)
    for t in range(4):
        if t == 0:
            re, ro = E[:, 0:128], O[:, 0:128]
        else:
            s = BSTARTS[t - 1] + 3
            re, ro = E[:, s:s + 128], O[:, s:s + 128]
        base = 16 + BLK * t
        Xline[:, :, base:base + 259] = re
        Xline[:, :, base + OBASE:base + OBASE + 259] = ro
    Xp = (Xline.reshape(ng_total, GROUP, 128, LINE)
          .transpose(0, 2, 1, 3).reshape(ng_total, 128, GROUP * LINE))
    # group bottom blocks: P-rows 488..514 (image rows 485..511)
    Xg = np.zeros((ng_total, 112, GLINE), FP8)
    Xg[:, :108, 16:16 + 259] = (
        E[:, 488:515].reshape(ng_total, GROUP * 27, 259))
    Xg[:, :108, 287:287 + 259] = (
        O[:, 488:515].reshape(ng_total, GROUP * 27, 259))
    ng = per // GROUP
    in_maps = [
        {"xp": Xp[i * ng:(i + 1) * ng],
         "xg": Xg[i * ng:(i + 1) * ng],
         "wt": wt, "bv": bv}
        for i in range(n_cores)
    ]
    return in_maps, per


def kernel(X, K, b):
    in_maps, per = _prep_inputs(X, K, b)
    nc = _get_module(per)
    res = run_bass_kernel_spmd(nc, in_maps, list(range(N_CORES)))
    outm = np.concatenate([res.results[i]["outm"] for i in range(N_CORES)],
                          axis=0)  # [n, 128, 2048] fp16
    outb = np.concatenate([res.results[i]["outb"] for i in range(N_CORES)],
                          axis=0)  # [ng, 96, 512] fp16
    n_total = outm.shape[0]
    ng = n_total // GROUP
    full = np.empty((n_total, H, W), np.float32)
    # outm[p, m, P, par, c, q] -> plane p, row 122*(2P+c)+m, col 2q+par
    o = outm[:, :KM].reshape(n_total, KM, 2, 2, 2, 256)
    full[:, :4 * KM] = (o.transpose(0, 2, 4, 1, 5, 3)
                        .reshape(n_total, 4 * KM, W).astype(np.float32))
    # outb[i, g, m, par, q] -> plane 4i+g, row 488+m, col 2q+par
    ob = outb.reshape(ng, GROUP, 24, 2, 256)
    full[:, 4 * KM:] = (ob.transpose(0, 1, 2, 4, 3)
                        .reshape(n_total, 24, W).astype(np.float32))
    return full.reshape(np.asarray(X).shape)
